# revision 9
# baseline (speedup 1.0000x reference)
"""MoE (noisy top-2 routing, dense expert stack) on 8 Trainium2 NeuronCores.

Strategy: expert-parallel with host-side routing as the sharding step. The
host computes the noisy gating in fp64 (bit-robust reproduction of the
reference's fp32 top-2 selection), ships each core exactly the tokens routed
to its expert (padded to a uniform tile count so all 8 cores run the same
SPMD program), and also ships the per-token top-2 softmax combine weight —
so the device runs nothing but the expert FFN.

The FFN runs on the PE array in fp8e4 DoubleRow mode (0.5 cycles/row, 256-
deep contraction per instruction = 4 MAC/PE/cycle, 2x the fp32r rate).
Full fp32-grade accuracy is recovered with a hi/lo split: every operand T is
represented as T ~= T_hi + T_lo (both fp8e4, T_lo the requantized residual,
~8 effective mantissa bits), and each matmul computes the three significant
cross terms hi*hi + hi*lo + lo*hi. That is 12 DoubleRow instructions per
1024-contraction where fp32r needs 8 full-rate instructions: 0.75x the
cycles at ~0.2% end-to-end error (gate is 2e-2). Weights are pre-scaled by
32 on the host so W values sit in e4m3's normal range (sigma ~ 1); the 1/32
is folded back in the (free) activation scale and the output epilogue.

Layer 1 emits h transposed (h-major) straight into SBUF as fp8 hi/lo, so it
chains into layer 2 as the stationary operand with no transpose. The host
scatter-adds the (at most 2) pre-weighted output rows per token — the
"all-reduce of the weighted combine" of the expert-parallel sharding, done
as part of unsharding. Per-core compute is the routed ~2/8 of the dense
reference instead of all 8 experts on all tokens.
"""

import sys

sys.path.insert(0, "/opt/trn_rl_repo")

import numpy as np

import concourse.bass as bass
import concourse.mybir as mybir
import concourse.tile as tile
from concourse import bacc
from concourse.bass_utils import run_bass_kernel_spmd

N_CORES = 8
N, D, H, E = 8192, 1024, 2048, 8
P = 128
KD = D // P                 # 8  k-chunks over D
MH = H // P                 # 16 h-chunks

F32 = mybir.dt.float32
F8 = mybir.dt.float8e4
ALU = mybir.AluOpType
ACT_F = mybir.ActivationFunctionType
DR = mybir.MatmulPerfMode.DoubleRow

WS = 32.0                   # host-side weight pre-scale (exact power of 2)


def _build(slots, repeat=1):
    """SPMD program for one core = one expert over `slots` routed tokens."""
    # Tile widths: full 512-wide tiles plus one 256/384-wide remainder tile.
    assert slots % P == 0 and slots % 512 in (0, 256, 384)
    widths = [512] * (slots // 512)
    if slots % 512:
        widths.append(slots % 512)

    nc = bacc.Bacc(None, target_bir_lowering=False, debug=False)

    xhT = nc.dram_tensor("xhT", [D, slots], F8, kind="ExternalInput")
    xlT = nc.dram_tensor("xlT", [D, slots], F8, kind="ExternalInput")
    w1h = nc.dram_tensor("w1h", [D, H], F8, kind="ExternalInput")
    w1l = nc.dram_tensor("w1l", [D, H], F8, kind="ExternalInput")
    w2h = nc.dram_tensor("w2h", [H, D], F8, kind="ExternalInput")
    w2l = nc.dram_tensor("w2l", [H, D], F8, kind="ExternalInput")
    b1c = nc.dram_tensor("b1c", [H], F32, kind="ExternalInput")
    b2c = nc.dram_tensor("b2c", [D], F32, kind="ExternalInput")
    wvd = nc.dram_tensor("wvd", [slots], F32, kind="ExternalInput")
    yc = nc.dram_tensor("yc", [slots, D], F32, kind="ExternalOutput")

    with tile.TileContext(nc) as tc:
        with (
            tc.tile_pool(name="persist", bufs=1) as persist,
            tc.tile_pool(name="xs", bufs=2) as xs,
            tc.tile_pool(name="hs", bufs=2) as hs,
            tc.tile_pool(name="h32s", bufs=3) as h32s,
            tc.tile_pool(name="yws", bufs=3) as yws,
            tc.tile_pool(name="ph", bufs=3, space="PSUM") as ph,
            tc.tile_pool(name="py", bufs=4, space="PSUM") as py,
        ):
            # ---- persistent tiles ----
            # Startup ordering: tile-0's x and W1's first quarter go first on
            # the SP queue (they gate the first layer-1 matmul); everything
            # needed later (W2, biases, combine weights) rides the Activation
            # HWDGE queue in parallel.
            def load_x(xtile, ltile, ss, TW):
                nc.sync.dma_start(
                    xtile[:, :, :TW],
                    xhT[:, ss].rearrange("(kd p) t -> p kd t", p=P),
                )
                nc.sync.dma_start(
                    ltile[:, :, :TW],
                    xlT[:, ss].rearrange("(kd p) t -> p kd t", p=P),
                )

            def x_tiles():
                xh = xs.tile([P, KD, 512], F8, tag="xh", name="xh")
                xl = xs.tile([P, KD, 512], F8, tag="xl", name="xl")
                return xh, xl

            # W1 is four column-quarter tiles per hi/lo half so the first
            # layer-1 matmuls depend only on the first quarter's DMA.
            W1h_sb = [
                persist.tile([P, KD, H // 4], F8, name=f"W1h{q}") for q in range(4)
            ]
            W1l_sb = [
                persist.tile([P, KD, H // 4], F8, name=f"W1l{q}") for q in range(4)
            ]
            # Queue discipline (a consumer waits for every DMA issued earlier
            # on the same engine): SP carries only x tiles, issued in
            # consumption order (next tile's prefetch goes out mid-L2, after
            # the current tile's first y group). Activation carries the
            # persistent tensors in first-use order, then y stores.
            cur = x_tiles()
            load_x(*cur, slice(0, widths[0]), widths[0])
            b1_sb = persist.tile([P, MH], F32)
            nc.scalar.dma_start(b1_sb[:], b1c.rearrange("(m p) -> p m", p=P))
            for q in range(4):
                qs = slice(q * (H // 4), (q + 1) * (H // 4))
                nc.scalar.dma_start(
                    W1h_sb[q][:], w1h[:, qs].rearrange("(kd p) h -> p kd h", p=P)
                )
                nc.scalar.dma_start(
                    W1l_sb[q][:], w1l[:, qs].rearrange("(kd p) h -> p kd h", p=P)
                )
            b2r = persist.tile([P, D], F32)
            nc.scalar.dma_start(b2r[:], b2c[None, :].to_broadcast((P, D)))
            wcol = persist.tile([P, slots // P], F32)
            nc.scalar.dma_start(wcol[:], wvd.rearrange("(c p) -> p c", p=P))
            # W2 as four nh-half tiles, loaded in layer-2 consumption order.
            W2h_sb = [
                persist.tile([P, MH, D // 2], F8, name=f"W2h{i}") for i in range(2)
            ]
            W2l_sb = [
                persist.tile([P, MH, D // 2], F8, name=f"W2l{i}") for i in range(2)
            ]
            for i in range(2):
                ns = slice(i * (D // 2), (i + 1) * (D // 2))
                nc.scalar.dma_start(
                    W2h_sb[i][:], w2h[:, ns].rearrange("(kh p) d -> p kh d", p=P)
                )
                nc.scalar.dma_start(
                    W2l_sb[i][:], w2l[:, ns].rearrange("(kh p) d -> p kh d", p=P)
                )

            nt = len(widths)
            for _rep in range(repeat):
                for ti, TW in enumerate(widths):
                    base = sum(widths[:ti])
                    nch = TW // P
                    xh, xl = cur
                    # prefetch the next tile's x (wraps to tile 0 under
                    # repeat>1 so the steady-state body is uniform); the DMA
                    # itself is issued mid-L2 below.
                    nti = (ti + 1) % nt
                    nxt = None
                    if _rep < repeat - 1 or ti < nt - 1:
                        nxt = x_tiles()

                    # layer 1: hT = relu((W1^T @ x)/32 + b1), h on partitions,
                    # split to fp8 hi/lo for layer 2. hT is physically two
                    # half-tensors (m 0-7 / 8-15) so layer 2's first k-pair
                    # group only depends on the first half.
                    hth = [
                        hs.tile([P, MH // 2, 512], F8, tag=f"hth{i}", name=f"hth{i}")
                        for i in range(2)
                    ]
                    htl = [
                        hs.tile([P, MH // 2, 512], F8, tag=f"htl{i}", name=f"htl{i}")
                        for i in range(2)
                    ]
                    for m in range(MH):
                        h_ps = ph.tile([P, 512], F32, tag="hps")
                        ms = slice((m % 4) * P, (m % 4 + 1) * P)
                        k = 0
                        for A, B in ((W1h_sb, xh), (W1l_sb, xh), (W1h_sb, xl)):
                            Aq = A[m // 4]
                            for j in range(KD // 2):
                                js = slice(2 * j, 2 * j + 2)
                                nc.tensor.matmul(
                                    h_ps[:, :TW],
                                    Aq[:, js, ms],
                                    B[:, js, :TW],
                                    start=(k == 0),
                                    stop=(k == 11),
                                    perf_mode=DR,
                                )
                                k += 1
                        h32 = h32s.tile([P, 512], F32, tag="h32")
                        nc.scalar.activation(
                            h32[:, :TW],
                            h_ps[:, :TW],
                            ACT_F.Relu,
                            bias=b1_sb[:, m : m + 1],
                            scale=1.0 / WS,
                        )
                        hv, mv = hth[m // 8], m % 8
                        nc.scalar.activation(
                            hv[:, mv, :TW], h32[:, :TW], ACT_F.Copy
                        )
                        nc.vector.tensor_tensor(
                            htl[m // 8][:, mv, :TW],
                            h32[:, :TW],
                            hv[:, mv, :TW],
                            ALU.subtract,
                        )

                    # layer 2: y = (hT^T @ W2)/32 + b2, then scale rows by the
                    # host-computed top-2 softmax weight and store.
                    for gi in range(2 * nch):
                        nh, c4 = gi // nch, gi % nch
                        ns = slice(nh * 512, (nh + 1) * 512)
                        cs = slice(c4 * P, (c4 + 1) * P)
                        y_ps = py.tile([P, 512], F32, tag="yps")
                        k = 0
                        for Sa, Wb in (
                            (hth, W2h_sb),
                            (htl, W2h_sb),
                            (hth, W2l_sb),
                        ):
                            for j in range(MH // 2):
                                jv = Sa[j // 4]
                                jls = slice(2 * (j % 4), 2 * (j % 4) + 2)
                                nc.tensor.matmul(
                                    y_ps[:],
                                    jv[:, jls, cs],
                                    Wb[nh][:, 2 * j : 2 * j + 2, :],
                                    start=(k == 0),
                                    stop=(k == 23),
                                    perf_mode=DR,
                                )
                                k += 1
                        if gi == 0 and nxt is not None:
                            nbase = sum(widths[:nti])
                            load_x(
                                *nxt,
                                slice(nbase, nbase + widths[nti]),
                                widths[nti],
                            )
                        ch = base // P + c4
                        yw = yws.tile([P, 512], F32, tag="yw")
                        nc.vector.scalar_tensor_tensor(
                            yw[:], y_ps[:], 1.0 / WS, b2r[:, ns],
                            ALU.mult, ALU.add,
                        )
                        nc.vector.tensor_scalar(
                            yw[:], yw[:], wcol[:, ch : ch + 1], None, ALU.mult
                        )
                        st_eng = nc.sync if gi % 2 else nc.scalar
                        st_eng.dma_start(
                            yc[base + c4 * P : base + (c4 + 1) * P, ns],
                            yw[:],
                        )
                    if nxt is not None:
                        cur = nxt

    nc.compile()
    return nc


_NC_CACHE = {}


def _get_nc(slots, repeat=1):
    key = (slots, repeat)
    if key not in _NC_CACHE:
        _NC_CACHE[key] = _build(slots, repeat)
    return _NC_CACHE[key]


def _split8(a):
    """fp8e4 hi/lo decomposition of a float32 array."""
    e4 = mybir.dt.np(F8)
    hi = a.astype(e4)
    lo = (a - hi.astype(np.float32)).astype(e4)
    return hi, lo


def prepare(x, W1, b1, W2, b2, Wg, bg, noise):
    """Host-side routing/sharding: fp64 noisy top-2, per-expert token lists,
    fp8 hi/lo quantization, per-core input maps, and the scatter-add spec."""
    x = np.ascontiguousarray(np.asarray(x, dtype=np.float32))
    noise = np.asarray(noise, dtype=np.float32)
    W1 = np.asarray(W1, dtype=np.float32)
    b1 = np.asarray(b1, dtype=np.float32)
    W2 = np.asarray(W2, dtype=np.float32)
    b2 = np.asarray(b2, dtype=np.float32)
    Wg = np.asarray(Wg, dtype=np.float32)
    bg = np.asarray(bg, dtype=np.float32)

    noisy = (
        x.astype(np.float64) @ Wg.astype(np.float64)
        + bg.astype(np.float64)
        + 0.1 * noise.astype(np.float64)
    )
    top2 = np.argsort(-noisy, axis=1)[:, :2]

    tok_lists = [np.nonzero((top2 == e).any(axis=1))[0] for e in range(E)]
    max_count = max(len(t) for t in tok_lists)
    slots = ((max_count + P - 1) // P) * P
    if slots % 512 == P:
        slots += P

    xq_hi, xq_lo = _split8(x)

    in_maps = []
    gathers = []
    for e in range(E):
        toks = tok_lists[e]
        cnt = len(toks)
        padded = np.zeros(slots, dtype=np.int64)
        padded[:cnt] = toks
        wv = np.zeros(slots, dtype=np.float32)
        if cnt:
            other = np.where(top2[toks, 0] == e, top2[toks, 1], top2[toks, 0])
            diff = noisy[toks, e] - noisy[toks, other]
            wv[:cnt] = (1.0 / (1.0 + np.exp(-diff))).astype(np.float32)
        w1hi, w1lo = _split8(W1[e] * WS)
        w2hi, w2lo = _split8(W2[e] * WS)
        in_maps.append(
            {
                "xhT": np.ascontiguousarray(xq_hi[padded].T),
                "xlT": np.ascontiguousarray(xq_lo[padded].T),
                "w1h": w1hi,
                "w1l": w1lo,
                "w2h": w2hi,
                "w2l": w2lo,
                "b1c": np.ascontiguousarray(b1[e]),
                "b2c": np.ascontiguousarray(b2[e]),
                "wvd": wv,
            }
        )
        gathers.append(toks)
    return in_maps, gathers, slots


def combine(results, gathers):
    """Unshard: scatter-add each core's pre-weighted rows into the output."""
    out = np.zeros((N, D), dtype=np.float32)
    for e in range(E):
        toks = gathers[e]
        out[toks] += results[e]["yc"][: len(toks)]
    return out


def kernel(x, W1, b1, W2, b2, Wg, bg, noise, **_ignored):
    in_maps, gathers, slots = prepare(x, W1, b1, W2, b2, Wg, bg, noise)
    nc = _get_nc(slots)
    res = run_bass_kernel_spmd(nc, in_maps, core_ids=list(range(N_CORES)))
    return combine(res.results, gathers)


# revision 10
# speedup vs baseline: 1.7879x; 1.7879x over previous
"""MoE (noisy top-2 routing, dense expert stack) on 8 Trainium2 NeuronCores.

Strategy: expert-parallel with host-side routing as the sharding step. The
host computes the noisy gating in fp64 (bit-robust reproduction of the
reference's fp32 top-2 selection), ships each core exactly the tokens routed
to its expert (padded to a uniform tile count so all 8 cores run the same
SPMD program), plus the per-token top-2 softmax combine weight — the device
runs nothing but the expert FFN, in fp16 (fp16 inputs, fp32 PSUM
accumulation: ~4e-4 end-to-end error against the fp32 reference).

Both weight matrices live in SBUF for the whole kernel (fp16 halves their
footprint), so the only per-tile DMA traffic is the x tile in and the y tile
out. Layer 1 emits h transposed (h-major) straight into SBUF as fp16, so it
chains into layer 2 as the stationary operand with no transpose.

DMA queue discipline (a consumer waits for every DMA issued earlier on the
same engine queue): the SP queue carries only x tiles, issued in consumption
order — the next tile's prefetch goes out mid-layer-2, after the current
tile's first output group. The Activation HWDGE queue carries the persistent
tensors in first-use order, then alternates y stores with SP.

The host scatter-adds the (at most 2) pre-weighted output rows per token —
the "all-reduce of the weighted combine" of the expert-parallel sharding,
done as part of unsharding. Per-core compute is the routed ~2/8 of the dense
reference instead of all 8 experts on all tokens.
"""

import sys

sys.path.insert(0, "/opt/trn_rl_repo")

import numpy as np

import concourse.bass as bass
import concourse.mybir as mybir
import concourse.tile as tile
from concourse import bacc
from concourse.bass_utils import run_bass_kernel_spmd

N_CORES = 8
N, D, H, E = 8192, 1024, 2048, 8
P = 128
KD = D // P                 # 8  k-chunks over D
MH = H // P                 # 16 h-chunks

F32 = mybir.dt.float32
F16 = mybir.dt.float16
ALU = mybir.AluOpType
ACT_F = mybir.ActivationFunctionType


def _build(slots, repeat=1):
    """SPMD program for one core = one expert over `slots` routed tokens."""
    assert slots % P == 0 and slots % 512 in (0, 256, 384)
    widths = [512] * (slots // 512)
    if slots % 512:
        widths.append(slots % 512)

    nc = bacc.Bacc(None, target_bir_lowering=False, debug=False)

    xT = nc.dram_tensor("xT", [D, slots], F16, kind="ExternalInput")
    w1c = nc.dram_tensor("w1c", [D, H], F16, kind="ExternalInput")
    w2c = nc.dram_tensor("w2c", [H, D], F16, kind="ExternalInput")
    b1c = nc.dram_tensor("b1c", [H], F32, kind="ExternalInput")
    b2c = nc.dram_tensor("b2c", [D], F32, kind="ExternalInput")
    wvd = nc.dram_tensor("wvd", [slots], F32, kind="ExternalInput")
    yc = nc.dram_tensor("yc", [slots, D], F32, kind="ExternalOutput")

    with tile.TileContext(nc) as tc:
        with (
            tc.tile_pool(name="persist", bufs=1) as persist,
            tc.tile_pool(name="xs", bufs=2) as xs,
            tc.tile_pool(name="hs", bufs=2) as hs,
            tc.tile_pool(name="yws", bufs=3) as yws,
            tc.tile_pool(name="ph", bufs=3, space="PSUM") as ph,
            tc.tile_pool(name="py", bufs=4, space="PSUM") as py,
        ):
            def x_tile():
                return xs.tile([P, KD, 512], F16, tag="xg", name="xg")

            def load_x(xtile, ss, TW):
                nc.sync.dma_start(
                    xtile[:, :, :TW],
                    xT[:, ss].rearrange("(kd p) t -> p kd t", p=P),
                )

            cur = x_tile()
            load_x(cur, slice(0, widths[0]), widths[0])
            b1_sb = persist.tile([P, MH], F32)
            nc.scalar.dma_start(b1_sb[:], b1c.rearrange("(m p) -> p m", p=P))
            # W1 as four column-quarter tiles so the first layer-1 matmuls
            # depend only on the first quarter's DMA.
            W1_sb = [
                persist.tile([P, KD, H // 4], F16, name=f"W1q{q}") for q in range(4)
            ]
            for q in range(4):
                qs = slice(q * (H // 4), (q + 1) * (H // 4))
                nc.scalar.dma_start(
                    W1_sb[q][:], w1c[:, qs].rearrange("(kd p) h -> p kd h", p=P)
                )
            b2r = persist.tile([P, D], F32)
            nc.scalar.dma_start(b2r[:], b2c[None, :].to_broadcast((P, D)))
            wcol = persist.tile([P, slots // P], F32)
            nc.scalar.dma_start(wcol[:], wvd.rearrange("(c p) -> p c", p=P))
            # W2 as two nh-half tiles, in layer-2 consumption order.
            W2_sb = [
                persist.tile([P, MH, D // 2], F16, name=f"W2h{i}") for i in range(2)
            ]
            for i in range(2):
                ns = slice(i * (D // 2), (i + 1) * (D // 2))
                nc.scalar.dma_start(
                    W2_sb[i][:], w2c[:, ns].rearrange("(kh p) d -> p kh d", p=P)
                )

            nt = len(widths)
            for _rep in range(repeat):
                for ti, TW in enumerate(widths):
                    base = sum(widths[:ti])
                    nch = TW // P
                    xg = cur
                    nti = (ti + 1) % nt
                    nxt = None
                    if _rep < repeat - 1 or ti < nt - 1:
                        nxt = x_tile()

                    # layer 1: hT = relu(W1^T @ x + b1), h on partitions,
                    # cast to fp16 by the activation itself. hT is two
                    # half-tensors (m 0-7 / 8-15) so layer 2's first k-chunks
                    # only depend on the first half.
                    hth = [
                        hs.tile([P, MH // 2, 512], F16, tag=f"hth{i}", name=f"hth{i}")
                        for i in range(2)
                    ]
                    for m in range(MH):
                        h_ps = ph.tile([P, 512], F32, tag="hps")
                        ms = slice((m % 4) * P, (m % 4 + 1) * P)
                        for kd in range(KD):
                            nc.tensor.matmul(
                                h_ps[:, :TW],
                                W1_sb[m // 4][:, kd, ms],
                                xg[:, kd, :TW],
                                start=(kd == 0),
                                stop=(kd == KD - 1),
                            )
                        nc.scalar.activation(
                            hth[m // 8][:, m % 8, :TW],
                            h_ps[:, :TW],
                            ACT_F.Relu,
                            bias=b1_sb[:, m : m + 1],
                        )

                    # layer 2: y = hT^T @ W2 + b2, then scale rows by the
                    # host-computed top-2 softmax weight and store.
                    for gi in range(2 * nch):
                        nh, c4 = gi // nch, gi % nch
                        ns = slice(nh * 512, (nh + 1) * 512)
                        cs = slice(c4 * P, (c4 + 1) * P)
                        y_ps = py.tile([P, 512], F32, tag="yps")
                        for kh in range(MH):
                            nc.tensor.matmul(
                                y_ps[:],
                                hth[kh // 8][:, kh % 8, cs],
                                W2_sb[nh][:, kh, :],
                                start=(kh == 0),
                                stop=(kh == MH - 1),
                            )
                        if gi == 0 and nxt is not None:
                            nbase = sum(widths[:nti])
                            load_x(nxt, slice(nbase, nbase + widths[nti]), widths[nti])
                        ch = base // P + c4
                        yw = yws.tile([P, 512], F32, tag="yw")
                        nc.vector.scalar_tensor_tensor(
                            yw[:], y_ps[:], 1.0, b2r[:, ns], ALU.mult, ALU.add
                        )
                        nc.vector.tensor_scalar(
                            yw[:], yw[:], wcol[:, ch : ch + 1], None, ALU.mult
                        )
                        st_eng = nc.sync if gi % 2 else nc.scalar
                        st_eng.dma_start(
                            yc[base + c4 * P : base + (c4 + 1) * P, ns],
                            yw[:],
                        )
                    if nxt is not None:
                        cur = nxt

    nc.compile()
    return nc


_NC_CACHE = {}


def _get_nc(slots, repeat=1):
    key = (slots, repeat)
    if key not in _NC_CACHE:
        _NC_CACHE[key] = _build(slots, repeat)
    return _NC_CACHE[key]


def prepare(x, W1, b1, W2, b2, Wg, bg, noise):
    """Host-side routing/sharding: fp64 noisy top-2, per-expert token lists,
    fp16 casts, per-core input maps, and the scatter-add spec."""
    x = np.ascontiguousarray(np.asarray(x, dtype=np.float32))
    noise = np.asarray(noise, dtype=np.float32)
    W1 = np.asarray(W1, dtype=np.float32)
    b1 = np.asarray(b1, dtype=np.float32)
    W2 = np.asarray(W2, dtype=np.float32)
    b2 = np.asarray(b2, dtype=np.float32)
    Wg = np.asarray(Wg, dtype=np.float32)
    bg = np.asarray(bg, dtype=np.float32)

    noisy = (
        x.astype(np.float64) @ Wg.astype(np.float64)
        + bg.astype(np.float64)
        + 0.1 * noise.astype(np.float64)
    )
    top2 = np.argsort(-noisy, axis=1)[:, :2]

    tok_lists = [np.nonzero((top2 == e).any(axis=1))[0] for e in range(E)]
    max_count = max(len(t) for t in tok_lists)
    slots = ((max_count + P - 1) // P) * P
    if slots % 512 == P:
        slots += P

    x16 = x.astype(np.float16)

    in_maps = []
    gathers = []
    for e in range(E):
        toks = tok_lists[e]
        cnt = len(toks)
        padded = np.zeros(slots, dtype=np.int64)
        padded[:cnt] = toks
        wv = np.zeros(slots, dtype=np.float32)
        if cnt:
            other = np.where(top2[toks, 0] == e, top2[toks, 1], top2[toks, 0])
            diff = noisy[toks, e] - noisy[toks, other]
            wv[:cnt] = (1.0 / (1.0 + np.exp(-diff))).astype(np.float32)
        in_maps.append(
            {
                "xT": np.ascontiguousarray(x16[padded].T),
                "w1c": W1[e].astype(np.float16),
                "w2c": W2[e].astype(np.float16),
                "b1c": np.ascontiguousarray(b1[e]),
                "b2c": np.ascontiguousarray(b2[e]),
                "wvd": wv,
            }
        )
        gathers.append(toks)
    return in_maps, gathers, slots


def combine(results, gathers):
    """Unshard: scatter-add each core's pre-weighted rows into the output."""
    out = np.zeros((N, D), dtype=np.float32)
    for e in range(E):
        toks = gathers[e]
        out[toks] += results[e]["yc"][: len(toks)]
    return out


def kernel(x, W1, b1, W2, b2, Wg, bg, noise, **_ignored):
    in_maps, gathers, slots = prepare(x, W1, b1, W2, b2, Wg, bg, noise)
    nc = _get_nc(slots)
    res = run_bass_kernel_spmd(nc, in_maps, core_ids=list(range(N_CORES)))
    return combine(res.results, gathers)
